# revision 1
# baseline (speedup 1.0000x reference)
"""Bilinear MoE-routing scores on 8 Trainium2 NeuronCores.

Problem: scores[n,k] = u[n,:] @ W_k @ v[n,:]; out[n] = sigmoid(scores[n, type_idx[n]]).
N=131072, D=256, K=8.

Sharding strategy: shard rows BY TYPE — core k gets exactly the rows whose
type_idx == k (so each core runs a single plain bilinear kernel with its own
W_k and never touches the other 7 weight matrices: 8x less matmul work than
the data-parallel all-K approach). The host groups rows by type (argsort),
pads each group to a common n_pad, uploads u transposed ([D, n_pad], so the
device needs no on-chip transpose for the matmul's contraction layout), and
scatters the per-core results back to the original row order.

Precision: fp16 hi/lo split ("fp16x3") for the matmul — u = uh + ul,
W = Wh + Wl (all fp16), t ≈ uh@Wh + ul@Wh + uh@Wl accumulated in fp32 PSUM.
This runs at 1 cycle/row on the PE (4x faster than fp32's 2-pass mode) while
keeping ~fp32 accuracy (validated: 5.4e-6 absmax on the sigmoid output vs
3.7e-6 for pure fp32). DMA bytes are unchanged (2+2 bytes vs 4).

Device kernel (per core, SPMD — per-core differences are pure data):
  t[n,e]  = sum_d uT[d,n] * W[d,e]        (TensorE, fp16x3, PSUM accumulate)
  s[n]    = sum_e t[n,e] * v[n,e]         (VectorE fused affine_mul_reduce)
  out[n]  = sigmoid(s[n])                 (ScalarE LUT)

Measured on HW (8 cores, paired-delta slope bench): ~107-118 us/invocation
(best uncontended passes ~99-107 us) vs a ~99 us pure-DMA floor for the same
33.9 MB/core at the practical ~341 GB/s HBM rate — i.e. at the memory roofline
modulo neighbor HBM contention. Naive data-parallel all-K fp32 compute would
be PE-bound at ~440+ us/core.
"""

import math

import numpy as np

P = 128  # SBUF partitions
D = 256  # hidden dim
N_CORES = 8
CHUNK = 1024  # max rows per DMA chunk
UBUFS = 4
VBUFS = 4
LADDER = True
G = 4  # row-tiles per PSUM super-tile ([128, G*256] = 2 banks)
REDUCE_ON_ACT = False
HAM_KEEPALIVE = False
AMR_BCAST = False

_PROGRAM_CACHE: dict = {}


def _chunk_sizes(n_pad: int):
    """Chunk ladder: small chunks at both ends for fast pipeline fill/drain,
    CHUNK-sized in the middle. All sizes are multiples of 128, sum == n_pad."""
    up = [256, 512] if LADDER else []
    down = [512, 256] if LADDER else []
    sizes = []
    rem = n_pad
    for s in up:
        if rem - s >= sum(down):
            sizes.append(s)
            rem -= s
    n_mid = max(0, (rem - sum(down)) // CHUNK)
    sizes += [CHUNK] * n_mid
    rem -= n_mid * CHUNK
    for s in down:
        if rem >= s:
            sizes.append(s)
            rem -= s
    while rem > 0:
        s = min(256, rem)
        sizes.append(s)
        rem -= s
    assert sum(sizes) == n_pad and all(s % P == 0 for s in sizes)
    return sizes


def _build_program(n_pad: int, repeat: int = 1):
    """Build + compile the SPMD Bass program for n_pad rows per core.

    repeat > 1 wraps the whole body in an on-device loop (benchmarking only
    — amortizes host/RPC overhead so wall-clock slope gives HW time/iter).
    """
    import contextlib

    import concourse.bass as bass  # noqa: F401
    import concourse.mybir as mybir
    import concourse.tile as tile
    from concourse import bacc

    f32 = mybir.dt.float32
    f16 = mybir.dt.float16
    n_tiles = n_pad // P
    assert n_pad % P == 0

    nc = bacc.Bacc(
        "TRN2", target_bir_lowering=False, debug=False, num_devices=N_CORES
    )
    # packed u: [s, D, n_pad] fp16, s=0 hi / s=1 lo
    u_t16 = nc.dram_tensor("u_t16", [2, D, n_pad], f16, kind="ExternalInput").ap()
    v = nc.dram_tensor("v", [n_pad, D], f32, kind="ExternalInput").ap()
    w_h = nc.dram_tensor("w_h", [D, D], f16, kind="ExternalInput").ap()
    w_l = nc.dram_tensor("w_l", [D, D], f16, kind="ExternalInput").ap()
    out = nc.dram_tensor("out", [n_pad], f32, kind="ExternalOutput").ap()

    with tile.TileContext(nc) as tc:
        with (
            tc.tile_pool(name="singles", bufs=1) as singles,
            tc.tile_pool(name="upool", bufs=UBUFS) as upool,
            tc.tile_pool(name="vpool", bufs=VBUFS) as vpool,
            tc.tile_pool(name="ppool", bufs=max(1, 8 // (G // 2)), space="PSUM") as ppool,
            tc.tile_pool(name="spool", bufs=4) as spool,
        ):
            rep_ctx = (
                tc.For_i(
                    0,
                    repeat,
                    1,
                    hint_engines=(
                        mybir.EngineType.PE,
                        mybir.EngineType.DVE,
                        mybir.EngineType.Activation,
                    ),
                )
                if repeat > 1
                else contextlib.nullcontext()
            )

            # s_buf[p, t] = score of padded row t*128+p
            s_buf = singles.tile([P, n_tiles], f32)
            bsink = singles.tile([P, 1], f32)
            sig_buf = singles.tile([P, n_tiles], f32)
            # W resident in SBUF: wh_sb[p, h, e] = Wh[h*128+p, e]
            wh_sb = singles.tile([P, 2, D], f16)
            wl_sb = singles.tile([P, 2, D], f16)
            nc.scalar.dma_start(out=wh_sb, in_=w_h.rearrange("(h p) e -> p h e", p=P))
            nc.scalar.dma_start(out=wl_sb, in_=w_l.rearrange("(h p) e -> p h e", p=P))

            with rep_ctx:
                c0 = 0
                for ch in _chunk_sizes(n_pad):
                    cht = ch // P
                    # u chunk: u_ch[p, s, h, n] = u_t16[s, h*128+p, c0+n]
                    # (s = hi/lo, h = contraction half) — one 4 MiB DMA
                    u_ch = upool.tile([P, 2, 2, ch], f16, tag="u")
                    nc.sync.dma_start(
                        out=u_ch,
                        in_=u_t16.rearrange("s (h p) n -> p s h n", p=P)[
                            :, :, :, c0 : c0 + ch
                        ],
                    )
                    # v chunk: v_ch[p, t, e] = v[c0 + t*128 + p, e]
                    # issued on the ACT HWDGE ring to parallelize descriptor gen
                    v_ch = vpool.tile([P, cht, D], f32, tag="v")
                    nc.scalar.dma_start(
                        out=v_ch,
                        in_=v[c0 : c0 + ch, :].rearrange("(t p) e -> p t e", p=P),
                    )
                    for st in range(0, cht, G):
                        g = min(G, cht - st)
                        ps = ppool.tile([P, g, D], f32, tag="ps")
                        if HAM_KEEPALIVE:
                            # tiny matmul on resident data: fires during the
                            # DMA wait so the PE HAM window never sees idle
                            nc.tensor.matmul(
                                ps[0:1, 0, 0:1], wh_sb[:, 0, 0:1],
                                wh_sb[:, 0, 0:1], start=True, stop=True,
                            )
                        for j in range(g):
                            t = st + j
                            sl = slice(t * P, (t + 1) * P)
                            # t = uh@Wh + uh@Wl + ul@Wh (fp32 accumulate)
                            nc.tensor.matmul(
                                ps[:, j, :], u_ch[:, 0, 0, sl], wh_sb[:, 0, :],
                                start=True, stop=False,
                            )
                            nc.tensor.matmul(
                                ps[:, j, :], u_ch[:, 0, 0, sl], wl_sb[:, 0, :],
                                start=False, stop=False,
                            )
                            nc.tensor.matmul(
                                ps[:, j, :], u_ch[:, 1, 0, sl], wh_sb[:, 0, :],
                                start=False, stop=False,
                            )
                            nc.tensor.matmul(
                                ps[:, j, :], u_ch[:, 0, 1, sl], wh_sb[:, 1, :],
                                start=False, stop=False,
                            )
                            nc.tensor.matmul(
                                ps[:, j, :], u_ch[:, 0, 1, sl], wl_sb[:, 1, :],
                                start=False, stop=False,
                            )
                            nc.tensor.matmul(
                                ps[:, j, :], u_ch[:, 1, 1, sl], wh_sb[:, 1, :],
                                start=False, stop=True,
                            )
                        gt = c0 // P + st
                        scr = spool.tile([P, g, D], f32, tag="scr")
                        # fused (t ⊙ v) + row-sum in one DVE custom op per tile
                        for j in range(g):
                            nc.vector.affine_mul_reduce(
                                out=(
                                    bsink.broadcast_to((P, D))
                                    if AMR_BCAST
                                    else scr[:, j, :]
                                ),
                                accum_out=s_buf[:, gt + j : gt + j + 1],
                                in0=ps[:, j, :],
                                in1=v_ch[:, st + j, :],
                                scale=1.0,
                                bias=0.0,
                            )
                    c0 += ch

                # incremental sigmoid + output: drain each ~quarter of the
                # score columns as soon as its chunks are reduced, so only the
                # last block's sigmoid+DMA sits in the kernel tail.
                out_pt = out.rearrange("(p t) -> p t", p=P)
                n_blk = 4
                bnd = [round(i * n_tiles / n_blk) for i in range(n_blk + 1)]
                for b0, b1 in zip(bnd[:-1], bnd[1:]):
                    if b1 > b0:
                        nc.scalar.activation(
                            out=sig_buf[:, b0:b1],
                            in_=s_buf[:, b0:b1],
                            func=mybir.ActivationFunctionType.Sigmoid,
                        )
                        # out[p*n_tiles + t] = sig_buf[p, t]; host unscrambles.
                        nc.sync.dma_start(
                            out=out_pt[:, b0:b1], in_=sig_buf[:, b0:b1]
                        )

    nc.compile()
    return nc


def _get_program(n_pad: int):
    if n_pad not in _PROGRAM_CACHE:
        _PROGRAM_CACHE[n_pad] = _build_program(n_pad)
    return _PROGRAM_CACHE[n_pad]


def _prep(u, v, weights, type_idx):
    """Group rows by type, pad, split fp16 hi/lo, build per-core input maps."""
    u = np.ascontiguousarray(np.asarray(u, dtype=np.float32))
    v = np.ascontiguousarray(np.asarray(v, dtype=np.float32))
    weights = np.ascontiguousarray(np.asarray(weights, dtype=np.float32))
    ti = np.asarray(type_idx).astype(np.int64).ravel()

    n, d = u.shape
    k = weights.shape[0]
    assert d == D and k == N_CORES

    order = np.argsort(ti, kind="stable")
    counts = np.bincount(ti, minlength=k)
    offsets = np.concatenate(([0], np.cumsum(counts)))
    n_pad = max(P, int(math.ceil(counts.max() / P)) * P)

    u_hi = u.astype(np.float16)
    u_lo = (u - u_hi.astype(np.float32)).astype(np.float16)

    in_maps = []
    core_rows = []
    for c in range(N_CORES):
        rows = order[offsets[c] : offsets[c + 1]]
        core_rows.append(rows)
        cnt = len(rows)
        ut16 = np.zeros((2, D, n_pad), dtype=np.float16)
        ut16[0, :, :cnt] = u_hi[rows].T
        ut16[1, :, :cnt] = u_lo[rows].T
        v_c = np.zeros((n_pad, D), dtype=np.float32)
        v_c[:cnt] = v[rows]
        wh = weights[c].astype(np.float16)
        wl = (weights[c] - wh.astype(np.float32)).astype(np.float16)
        in_maps.append({"u_t16": ut16, "v": v_c, "w_h": wh, "w_l": wl})
    return in_maps, core_rows, n_pad


def _run(u, v, weights, type_idx, trace=False):
    from concourse import bass_utils
    from concourse.bass_interp import get_hw_module

    n = np.asarray(u).shape[0]
    in_maps, core_rows, n_pad = _prep(u, v, weights, type_idx)
    n_tiles = n_pad // P

    nc = _get_program(n_pad)
    old_m = nc.m
    nc.m = get_hw_module(nc.m)
    try:
        res = bass_utils.run_bass_kernel_spmd(
            nc, in_maps, core_ids=list(range(N_CORES)), trace=trace
        )
    finally:
        nc.m = old_m

    final = np.empty((n,), dtype=np.float32)
    for c in range(N_CORES):
        arr = np.asarray(res.results[c]["out"]).reshape(P, n_tiles)
        # device row m = t*128+p lives at arr[p, t]
        per_row = arr.T.reshape(-1)[: len(core_rows[c])]
        final[core_rows[c]] = per_row
    return final, res


def kernel(**inputs) -> np.ndarray:
    out, _ = _run(
        inputs["u_hidden"],
        inputs["v_hidden"],
        inputs["weights"],
        inputs["type_idx"],
        trace=False,
    )
    return out



# revision 2
# speedup vs baseline: 2.0740x; 2.0740x over previous
"""Bilinear MoE-routing scores on 8 Trainium2 NeuronCores.

Problem: scores[n,k] = u[n,:] @ W_k @ v[n,:]; out[n] = sigmoid(scores[n, type_idx[n]]).
N=131072, D=256, K=8.

Sharding strategy: shard rows BY TYPE — core k gets exactly the rows whose
type_idx == k (so each core runs a single plain bilinear kernel with its own
W_k and never touches the other 7 weight matrices: 8x less matmul work than
the data-parallel all-K approach). The host groups rows by type (argsort),
pads each group to a common n_pad, packs u (transposed) and v into
per-partition-contiguous fp16 chunk blocks (4 KB DMA lines), and scatters the
per-core results back to the original row order.

Precision: pure fp16 inputs (u, W, v all rounded to fp16; products exact in
fp32, PSUM/accumulation fp32). Host-side validation vs a float64 reference
puts the absmax output error at ~5.6e-3 — comfortably under the 2e-2 gate —
while halving HBM traffic vs the fp16 hi/lo split (17 MB/core vs 34 MB/core)
and cutting PE matmul work 3x (2 matmuls per 128-row tile instead of 6).

Device kernel (per core, SPMD — per-core differences are pure data):
  t[n,e]  = sum_d uT[d,n] * W[d,e]        (TensorE fp16, fp32 PSUM accumulate)
  s[n]    = sum_e t[n,e] * v[n,e]         (VectorE fused affine_mul_reduce)
  out[n]  = sigmoid(s[n])                 (ScalarE LUT)

Engine budget per core (n_pad ~ 16512, 129 row-tiles): DMA ~17 MB at the
practical ~341 GB/s rate = ~50 us; DVE 129 x ~390 ns = ~50 us; PE 129 x
~215 ns = ~28 us; ACT ~2 us. DMA and DVE tie at the roofline.
"""

import math

import numpy as np

P = 128  # SBUF partitions
D = 256  # hidden dim
N_CORES = 8
CHUNK = 1024  # max rows per DMA chunk
UBUFS = 4
VBUFS = 4
LADDER = True
G = 4  # row-tiles per PSUM super-tile ([128, G*256] = 2 banks)

_PROGRAM_CACHE: dict = {}


def _chunk_sizes(n_pad: int):
    """Chunk ladder: small chunks at both ends for fast pipeline fill/drain,
    CHUNK-sized in the middle. All sizes are multiples of 128, sum == n_pad."""
    up = [256, 512] if LADDER else []
    down = [512, 256] if LADDER else []
    sizes = []
    rem = n_pad
    for s in up:
        if rem - s >= sum(down):
            sizes.append(s)
            rem -= s
    n_mid = max(0, (rem - sum(down)) // CHUNK)
    sizes += [CHUNK] * n_mid
    rem -= n_mid * CHUNK
    for s in down:
        if rem >= s:
            sizes.append(s)
            rem -= s
    while rem > 0:
        s = min(256, rem)
        sizes.append(s)
        rem -= s
    assert sum(sizes) == n_pad and all(s % P == 0 for s in sizes)
    return sizes


def _build_program(n_pad: int, repeat: int = 1):
    """Build + compile the SPMD Bass program for n_pad rows per core.

    repeat > 1 wraps the whole body in an on-device loop (benchmarking only
    — amortizes host/RPC overhead so wall-clock slope gives HW time/iter).
    """
    import contextlib

    import concourse.bass as bass  # noqa: F401
    import concourse.mybir as mybir
    import concourse.tile as tile
    from concourse import bacc

    f32 = mybir.dt.float32
    f16 = mybir.dt.float16
    n_tiles = n_pad // P
    assert n_pad % P == 0

    nc = bacc.Bacc(
        "TRN2", target_bir_lowering=False, debug=False, num_devices=N_CORES
    )
    # u packed per chunk: for chunk (c0, ch), u16[p, 2*c0 + h*ch + i] =
    # uT[h*128+p, c0+i] — each partition's chunk block is one contiguous
    # 4 KB run (ch=1024).
    u16 = nc.dram_tensor("u16", [P, 2 * n_pad], f16, kind="ExternalInput").ap()
    # v packed p-major: v16[p, t*256 + e] = v[t*128 + p, e]; per-chunk
    # slices are contiguous per partition.
    v16 = nc.dram_tensor("v16", [P, n_tiles * D], f16, kind="ExternalInput").ap()
    w16 = nc.dram_tensor("w16", [D, D], f16, kind="ExternalInput").ap()
    out = nc.dram_tensor("out", [n_pad], f32, kind="ExternalOutput").ap()

    with tile.TileContext(nc) as tc:
        with (
            tc.tile_pool(name="singles", bufs=1) as singles,
            tc.tile_pool(name="upool", bufs=UBUFS) as upool,
            tc.tile_pool(name="vpool", bufs=VBUFS) as vpool,
            tc.tile_pool(name="ppool", bufs=max(1, 8 // (G // 2)), space="PSUM") as ppool,
            tc.tile_pool(name="spool", bufs=4) as spool,
        ):
            rep_ctx = (
                tc.For_i(
                    0,
                    repeat,
                    1,
                    hint_engines=(
                        mybir.EngineType.PE,
                        mybir.EngineType.DVE,
                        mybir.EngineType.Activation,
                    ),
                )
                if repeat > 1
                else contextlib.nullcontext()
            )

            # s_buf[p, t] = score of padded row t*128+p
            s_buf = singles.tile([P, n_tiles], f32)
            sig_buf = singles.tile([P, n_tiles], f32)
            # W resident in SBUF: w_sb[p, h, e] = W[h*128+p, e]
            w_sb = singles.tile([P, 2, D], f16)
            nc.scalar.dma_start(out=w_sb, in_=w16.rearrange("(h p) e -> p h e", p=P))

            with rep_ctx:
                c0 = 0
                for ch in _chunk_sizes(n_pad):
                    cht = ch // P
                    # u chunk: u_ch[p, h, n] = uT[h*128+p, c0+n] — one
                    # contiguous-per-partition DMA
                    u_ch = upool.tile([P, 2, ch], f16, tag="u")
                    nc.sync.dma_start(
                        out=u_ch,
                        in_=u16[:, 2 * c0 : 2 * (c0 + ch)].rearrange(
                            "p (h n) -> p h n", h=2
                        ),
                    )
                    # v chunk: v_ch[p, t, e] = v[c0 + t*128 + p, e]
                    # issued on the ACT HWDGE ring to parallelize descriptor gen
                    t0 = c0 // P
                    v_ch = vpool.tile([P, cht, D], f16, tag="v")
                    nc.scalar.dma_start(
                        out=v_ch,
                        in_=v16[:, t0 * D : (t0 + cht) * D].rearrange(
                            "p (t e) -> p t e", e=D
                        ),
                    )
                    for st in range(0, cht, G):
                        g = min(G, cht - st)
                        ps = ppool.tile([P, g, D], f32, tag="ps")
                        for j in range(g):
                            t = st + j
                            sl = slice(t * P, (t + 1) * P)
                            # t = u@W, contraction d split in two 128-halves
                            nc.tensor.matmul(
                                ps[:, j, :], u_ch[:, 0, sl], w_sb[:, 0, :],
                                start=True, stop=False,
                            )
                            nc.tensor.matmul(
                                ps[:, j, :], u_ch[:, 1, sl], w_sb[:, 1, :],
                                start=False, stop=True,
                            )
                        gt = c0 // P + st
                        scr = spool.tile([P, g, D], f16, tag="scr")
                        # fused (t ⊙ v) + row-sum in one DVE custom op per tile
                        for j in range(g):
                            nc.vector.affine_mul_reduce(
                                out=scr[:, j, :],
                                accum_out=s_buf[:, gt + j : gt + j + 1],
                                in0=ps[:, j, :],
                                in1=v_ch[:, st + j, :],
                                scale=1.0,
                                bias=0.0,
                            )
                    c0 += ch

                # incremental sigmoid + output: drain each ~quarter of the
                # score columns as soon as its chunks are reduced, so only the
                # last block's sigmoid+DMA sits in the kernel tail.
                out_pt = out.rearrange("(p t) -> p t", p=P)
                n_blk = 4
                bnd = [round(i * n_tiles / n_blk) for i in range(n_blk + 1)]
                for b0, b1 in zip(bnd[:-1], bnd[1:]):
                    if b1 > b0:
                        nc.scalar.activation(
                            out=sig_buf[:, b0:b1],
                            in_=s_buf[:, b0:b1],
                            func=mybir.ActivationFunctionType.Sigmoid,
                        )
                        # out[p*n_tiles + t] = sig_buf[p, t]; host unscrambles.
                        nc.sync.dma_start(
                            out=out_pt[:, b0:b1], in_=sig_buf[:, b0:b1]
                        )

    nc.compile()
    return nc


def _get_program(n_pad: int):
    if n_pad not in _PROGRAM_CACHE:
        _PROGRAM_CACHE[n_pad] = _build_program(n_pad)
    return _PROGRAM_CACHE[n_pad]


def _prep(u, v, weights, type_idx):
    """Group rows by type, pad, round to fp16, pack per-partition-contiguous
    chunk blocks, build per-core input maps."""
    u = np.ascontiguousarray(np.asarray(u, dtype=np.float32))
    v = np.ascontiguousarray(np.asarray(v, dtype=np.float32))
    weights = np.ascontiguousarray(np.asarray(weights, dtype=np.float32))
    ti = np.asarray(type_idx).astype(np.int64).ravel()

    n, d = u.shape
    k = weights.shape[0]
    assert d == D and k == N_CORES

    order = np.argsort(ti, kind="stable")
    counts = np.bincount(ti, minlength=k)
    offsets = np.concatenate(([0], np.cumsum(counts)))
    n_pad = max(P, int(math.ceil(counts.max() / P)) * P)
    n_tiles = n_pad // P
    chunks = _chunk_sizes(n_pad)

    u_hi = u.astype(np.float16)
    v_hi = v.astype(np.float16)

    in_maps = []
    core_rows = []
    for c in range(N_CORES):
        rows = order[offsets[c] : offsets[c + 1]]
        core_rows.append(rows)
        cnt = len(rows)
        # uT padded: [D, n_pad]
        uT = np.zeros((D, n_pad), dtype=np.float16)
        uT[:, :cnt] = u_hi[rows].T
        u_pack = np.empty((P, 2 * n_pad), dtype=np.float16)
        c0 = 0
        for ch in chunks:
            blk = uT[:, c0 : c0 + ch].reshape(2, P, ch)
            u_pack[:, 2 * c0 : 2 * (c0 + ch)] = blk.transpose(1, 0, 2).reshape(
                P, 2 * ch
            )
            c0 += ch
        # v p-major: v_pack[p, t*D + e] = v[rows[t*128+p], e]
        v_c = np.zeros((n_pad, D), dtype=np.float16)
        v_c[:cnt] = v_hi[rows]
        v_pack = np.ascontiguousarray(
            v_c.reshape(n_tiles, P, D).transpose(1, 0, 2)
        ).reshape(P, n_tiles * D)
        w_c = weights[c].astype(np.float16)
        in_maps.append({"u16": u_pack, "v16": v_pack, "w16": w_c})
    return in_maps, core_rows, n_pad


def _run(u, v, weights, type_idx, trace=False):
    from concourse import bass_utils
    from concourse.bass_interp import get_hw_module

    n = np.asarray(u).shape[0]
    in_maps, core_rows, n_pad = _prep(u, v, weights, type_idx)
    n_tiles = n_pad // P

    nc = _get_program(n_pad)
    old_m = nc.m
    nc.m = get_hw_module(nc.m)
    try:
        res = bass_utils.run_bass_kernel_spmd(
            nc, in_maps, core_ids=list(range(N_CORES)), trace=trace
        )
    finally:
        nc.m = old_m

    final = np.empty((n,), dtype=np.float32)
    for c in range(N_CORES):
        arr = np.asarray(res.results[c]["out"]).reshape(P, n_tiles)
        # device row m = t*128+p lives at arr[p, t]
        per_row = arr.T.reshape(-1)[: len(core_rows[c])]
        final[core_rows[c]] = per_row
    return final, res


def kernel(**inputs) -> np.ndarray:
    out, _ = _run(
        inputs["u_hidden"],
        inputs["v_hidden"],
        inputs["weights"],
        inputs["type_idx"],
        trace=False,
    )
    return out
